# revision 1
# baseline (speedup 1.0000x reference)
"""Multi-head attention (B=4, S=2048, D=1024, H=16) on 8 trn2 NeuronCores.

Sharding: (batch x query-half) -> 8 shards, no collectives. Each core computes
K/V projections for its batch (2x redundant), Q projection + attention +
output projection for its 1024 query rows.

All matmuls run in float32r (TF32-like, ~1e-4 rel err), fp32 accumulation.
"""
import sys
sys.path.insert(0, '/opt/trn_rl_repo')
import numpy as np
import concourse.bass as bass
from concourse import bacc
import concourse.mybir as mybir
import concourse.tile as tile
from concourse.bass_utils import run_bass_kernel_spmd

dt = mybir.dt
F = mybir.ActivationFunctionType

B, S, D, H = 4, 2048, 1024, 16
DK = D // H          # 64
QR = S // 2          # 1024 query rows per core
NC = 8               # cores
DC = D // 128        # 8 d-chunks
KC = S // 128        # 16 k-chunks
G = 65               # v_aug group width (64 v cols + ones col)

_nc_cache = None


def build_nc(phases=("q", "k", "v", "a", "o")):
    nc = bacc.Bacc()
    qT_in = nc.dram_tensor("qT_in", [D, QR], dt.float32, kind="ExternalInput")
    kT_in = nc.dram_tensor("kT_in", [D, S], dt.float32, kind="ExternalInput")
    vT_in = nc.dram_tensor("vT_in", [D, S], dt.float32, kind="ExternalInput")
    WqT = nc.dram_tensor("WqT", [D, D], dt.float32, kind="ExternalInput")
    WkT = nc.dram_tensor("WkT", [D, D], dt.float32, kind="ExternalInput")
    WvT = nc.dram_tensor("WvT", [D, D], dt.float32, kind="ExternalInput")
    WoR = nc.dram_tensor("WoR", [DK, H, D], dt.float32, kind="ExternalInput")
    bq_pf = nc.dram_tensor("bq_pf", [128, DC], dt.float32, kind="ExternalInput")
    bk_pf = nc.dram_tensor("bk_pf", [128, DC], dt.float32, kind="ExternalInput")
    bv_pj = nc.dram_tensor("bv_pj", [DK, H], dt.float32, kind="ExternalInput")
    bo_row = nc.dram_tensor("bo_row", [1, D], dt.float32, kind="ExternalInput")
    y_out = nc.dram_tensor("y_out", [QR, D], dt.float32, kind="ExternalOutput")

    f32r = dt.float32r
    with tile.TileContext(nc) as tc:
      with tc.tile_pool(name="dramp", bufs=1, space="DRAM") as dramp:
        xh_spill = dramp.tile([DK, H, QR], dt.float32r)
        with tc.tile_pool(name="persist", bufs=1) as persist:
            qT_sb = persist.tile([128, DC, QR], f32r)       # 32 KB/part
            kT_sb = persist.tile([128, DC, S], f32r)        # 64 KB/part

            # ---- Wk tile allocated below phase-Q pools; DMA emitted mid-phase-Q ----
            pwk_cm = tc.tile_pool(name="pwk", bufs=1)
            pwk = pwk_cm.__enter__()
            wk = pwk.tile([128, DC, D], f32r)

            # ---- Phase Q: qT = (query @ Wq.T + bq).T, feat-major ----
            if "q" in phases:
             with tc.tile_pool(name="phq", bufs=1) as phq, \
                 tc.tile_pool(name="psq", bufs=4, space="PSUM") as psq:
                queryT = phq.tile([128, DC, QR], f32r)
                wq = phq.tile([128, DC, D], f32r)
                bq_sb = phq.tile([128, DC], dt.float32)
                qv = qT_in[:, :].rearrange("(c p) q -> p c q", p=128).bitcast(f32r)
                wqv = WqT[:, :].rearrange("(c p) f -> p c f", p=128).bitcast(f32r)
                nc.sync.dma_start(out=bq_sb, in_=bq_pf[:, :])
                nc.sync.dma_start(out=wq[:, :, 0:128], in_=wqv[:, :, 0:128])
                for rb in range(2):
                    nc.sync.dma_start(out=queryT[:, :, rb * 512:(rb + 1) * 512], in_=qv[:, :, rb * 512:(rb + 1) * 512])
                nc.sync.dma_start(out=wq[:, :, 128:1024], in_=wqv[:, :, 128:1024])
                for ft in range(DC):
                    if ft == 1:
                        nc.sync.dma_start(out=wk, in_=WkT[:, :].rearrange("(c p) f -> p c f", p=128).bitcast(f32r))
                    for rb in range(QR // 512):
                        ps = psq.tile([128, 512], dt.float32, tag="psq")
                        for dc in range(DC):
                            nc.tensor.matmul(out=ps, lhsT=wq[:, dc, ft * 128:(ft + 1) * 128],
                                             rhs=queryT[:, dc, rb * 512:(rb + 1) * 512],
                                             start=(dc == 0), stop=(dc == DC - 1))
                        nc.scalar.activation(out=qT_sb[:, ft, rb * 512:(rb + 1) * 512], in_=ps,
                                             func=F.Identity, bias=bq_sb[:, ft:ft + 1], scale=1.0)

            # ---- Phase K: kT = (key @ Wk.T + bk).T, feat-major ----
            if "k" in phases:
             with tc.tile_pool(name="phk", bufs=1) as phk, \
                 tc.tile_pool(name="phk2", bufs=2) as phk2, \
                 tc.tile_pool(name="psk", bufs=4, space="PSUM") as psk:
                bk_sb = phk.tile([128, DC], dt.float32)
                nc.sync.dma_start(out=bk_sb, in_=bk_pf[:, :])
                keyT_v = kT_in[:, :].rearrange("(c p) s -> p c s", p=128)
                for kb in range(S // 512):
                    keyb = phk2.tile([128, DC, 512], f32r, tag="keyb")
                    nc.sync.dma_start(out=keyb, in_=keyT_v[:, :, kb * 512:(kb + 1) * 512].bitcast(f32r))
                    for ft in range(DC):
                        ps = psk.tile([128, 512], dt.float32, tag="psk")
                        for dc in range(DC):
                            nc.tensor.matmul(out=ps, lhsT=wk[:, dc, ft * 128:(ft + 1) * 128],
                                             rhs=keyb[:, dc, :],
                                             start=(dc == 0), stop=(dc == DC - 1))
                        nc.scalar.activation(out=kT_sb[:, ft, kb * 512:(kb + 1) * 512], in_=ps,
                                             func=F.Identity, bias=bk_sb[:, ft:ft + 1], scale=1.0)

            # ---- Phase V: v_aug[k, h*G:(h*G+64)] = value @ Wv.T (bias applied later), col G-1 = 1 ----
            pwk_cm.__exit__(None, None, None)
            vaug_cm = tc.tile_pool(name="vaugp", bufs=1)
            vaug_pool = vaug_cm.__enter__()
            vaug_sb = vaug_pool.tile([128, KC, H * G], f32r)  # 65 KB/part
            if "v" in phases:
             with tc.tile_pool(name="phv", bufs=1) as phv, \
                 tc.tile_pool(name="phv2", bufs=3) as phv2, \
                 tc.tile_pool(name="psv", bufs=4, space="PSUM") as psv:
                vaug_g = vaug_sb.rearrange("p t (g c) -> p t g c", g=H)
                nc.vector.memset(vaug_g[:, :, :, 64:65].bitcast(dt.float32), 1.0)
                valT_v = vT_in[:, :].rearrange("(c p) s -> p c s", p=128)
                wv = phv.tile([128, DC, D], f32r)
                nc.sync.dma_start(out=wv, in_=WvT[:, :].rearrange("(c p) f -> p c f", p=128).bitcast(f32r))
                for rt in range(KC):
                    vtb = phv2.tile([128, DC, 128], f32r, tag="vtb")
                    nc.sync.dma_start(out=vtb, in_=valT_v[:, :, rt * 128:(rt + 1) * 128].bitcast(f32r))
                    for fb in range(2):
                        ps = psv.tile([128, 512], dt.float32, tag="psv")
                        for dc in range(DC):
                            nc.tensor.matmul(out=ps, lhsT=vtb[:, dc, :],
                                             rhs=wv[:, dc, fb * 512:(fb + 1) * 512],
                                             start=(dc == 0), stop=(dc == DC - 1))
                        nc.vector.tensor_copy(
                            out=vaug_g[:, rt, fb * 8:(fb + 1) * 8, 0:64],
                            in_=ps.rearrange("p (g c) -> p g c", g=8))

            # ---- Phase A: attention per head; xh = softmax(qk/8) @ v_aug, normalized + bv ----
            if "a" in phases:
             with tc.tile_pool(name="pha", bufs=1) as pha, \
                 tc.tile_pool(name="pha_es", bufs=4) as pha_es, \
                 tc.tile_pool(name="pha_xh", bufs=2) as pha_xh, \
                 tc.tile_pool(name="pha_sm", bufs=2) as pha_sm, \
                 tc.tile_pool(name="ps_sc", bufs=2, space="PSUM") as ps_sc, \
                 tc.tile_pool(name="ps_pv", bufs=1, space="PSUM") as ps_pv, \
                 tc.tile_pool(name="ps_bc", bufs=1, space="PSUM") as ps_bc:
                ones65 = pha.tile([65, 64], f32r)
                nc.vector.memset(ones65[64:65, :].bitcast(dt.float32), 1.0)
                bv_sb = pha.tile([DK, H], dt.float32)
                nc.sync.dma_start(out=bv_sb, in_=bv_pj[:, :])
                for h in range(H):
                    off = (h % 2) * 64
                    fc = h // 2
                    pvA = ps_pv.tile([65, 512], dt.float32, tag="pvA")
                    pvB = ps_pv.tile([65, 512], dt.float32, tag="pvB")
                    for kc in range(KC):
                        sc = ps_sc.tile([128, QR], dt.float32, tag="sc")
                        for qh in range(2):
                            nc.tensor.matmul(out=sc[:, qh * 512:(qh + 1) * 512],
                                             lhsT=kT_sb[off:off + 64, fc, kc * 128:(kc + 1) * 128],
                                             rhs=qT_sb[off:off + 64, fc, qh * 512:(qh + 1) * 512],
                                             start=True, stop=True)
                        es = pha_es.tile([128, QR], f32r, tag="es")
                        nc.scalar.activation(out=es, in_=sc, func=F.Exp, scale=0.125)
                        nc.tensor.matmul(out=pvA, lhsT=vaug_sb[:, kc, h * G:h * G + G],
                                         rhs=es[:, 0:512], start=(kc == 0), stop=(kc == KC - 1))
                        nc.tensor.matmul(out=pvB, lhsT=vaug_sb[:, kc, h * G:h * G + G],
                                         rhs=es[:, 512:1024], start=(kc == 0), stop=(kc == KC - 1))
                    xh = pha_xh.tile([DK, QR], f32r, tag="xh")
                    for qh, pv in ((0, pvA), (1, pvB)):
                        pv_sb = pha_sm.tile([65, 512], dt.float32, tag="pv_sb")
                        nc.vector.tensor_copy(out=pv_sb, in_=pv)
                        recip = pha_sm.tile([65, 512], f32r, tag="recip")
                        with nc.allow_low_precision(reason="f32r softmax normalizer"):
                            nc.vector.reciprocal(out=recip[64:65, :], in_=pv_sb[64:65, :])
                        bc = ps_bc.tile([64, 512], dt.float32, tag="bc")
                        nc.tensor.matmul(out=bc, lhsT=ones65[64:65, :], rhs=recip[64:65, :],
                                         start=True, stop=True)
                        bc_sb = pha_sm.tile([64, 512], dt.float32, tag="bc_sb")
                        nc.vector.tensor_copy(out=bc_sb, in_=bc)
                        nc.vector.tensor_mul(out=xh[:, qh * 512:(qh + 1) * 512],
                                             in0=pv_sb[0:64, :], in1=bc_sb)
                    with nc.allow_low_precision(reason="f32r attention output"):
                        nc.vector.tensor_scalar(out=xh, in0=xh, scalar1=bv_sb[:, h:h + 1],
                                                scalar2=None, op0=mybir.AluOpType.add)
                    nc.sync.dma_start(out=xh_spill[:, h, :], in_=xh)
            vaug_cm.__exit__(None, None, None)

        # ---- Phase O: y = concat_h(xh) @ Wo.T + bo ----
        if "o" in phases:
         with tc.tile_pool(name="pho", bufs=1) as pho, \
             tc.tile_pool(name="pho2", bufs=2) as pho2, \
             tc.tile_pool(name="ps_y", bufs=4, space="PSUM") as ps_y:
            wo = pho.tile([DK, H, D], f32r)
            nc.sync.dma_start(out=wo, in_=WoR[:, :, :].bitcast(f32r))
            ones1 = pho.tile([1, 128], f32r)
            nc.vector.memset(ones1.bitcast(dt.float32), 1.0)
            bo_sb = pho.tile([1, D], f32r)
            nc.sync.dma_start(out=bo_sb, in_=bo_row[:, :].bitcast(f32r))
            bo_bc = pho.tile([128, D], dt.float32)
            for nb in range(2):
                bps = ps_y.tile([128, 512], dt.float32, tag="bps")
                nc.tensor.matmul(out=bps, lhsT=ones1, rhs=bo_sb[:, nb * 512:(nb + 1) * 512],
                                 start=True, stop=True)
                nc.vector.tensor_copy(out=bo_bc[:, nb * 512:(nb + 1) * 512], in_=bps)
            for qs in range(QR // 128):
                xt = pho2.tile([DK, H, 128], f32r, tag="xt")
                nc.sync.dma_start(out=xt, in_=xh_spill[:, :, qs * 128:(qs + 1) * 128])
                for fb in range(2):
                    ps = ps_y.tile([128, 512], dt.float32, tag="ps_y")
                    for h in range(H):
                        nc.tensor.matmul(out=ps, lhsT=xt[:, h, :],
                                         rhs=wo[:, h, fb * 512:(fb + 1) * 512],
                                         start=(h == 0), stop=(h == H - 1))
                    ysb = pho2.tile([128, 512], dt.float32, tag="ysb")
                    nc.vector.tensor_add(out=ysb, in0=ps, in1=bo_bc[:, fb * 512:(fb + 1) * 512])
                    nc.sync.dma_start(out=y_out[qs * 128:(qs + 1) * 128, fb * 512:(fb + 1) * 512], in_=ysb)

    nc.finalize()
    return nc


def _get_nc():
    global _nc_cache
    if _nc_cache is None:
        _nc_cache = build_nc()
    return _nc_cache


def kernel(query, key_, value, mask, Wq, bq, Wk, bk, Wv, bv, Wo, bo):
    query = np.asarray(query, dtype=np.float32)
    key_ = np.asarray(key_, dtype=np.float32)
    value = np.asarray(value, dtype=np.float32)
    Wq = np.asarray(Wq, dtype=np.float32)
    bq = np.asarray(bq, dtype=np.float32)
    Wk = np.asarray(Wk, dtype=np.float32)
    bk = np.asarray(bk, dtype=np.float32)
    Wv = np.asarray(Wv, dtype=np.float32)
    bv = np.asarray(bv, dtype=np.float32)
    Wo = np.asarray(Wo, dtype=np.float32)
    bo = np.asarray(bo, dtype=np.float32)

    nc = _get_nc()

    WqT = np.ascontiguousarray(Wq.T)
    WkT = np.ascontiguousarray(Wk.T)
    WvT = np.ascontiguousarray(Wv.T)
    # WoR[j, h, f] = Wo.T[64h + j, f]
    WoR = np.ascontiguousarray(Wo.T.reshape(H, DK, D).transpose(1, 0, 2))
    bq_pf = np.ascontiguousarray(bq.reshape(DC, 128).T)
    bk_pf = np.ascontiguousarray(bk.reshape(DC, 128).T)
    bv_pj = np.ascontiguousarray(bv.reshape(H, DK).T)
    bo_row = bo.reshape(1, D)

    kT_b = [np.ascontiguousarray(key_[b].T) for b in range(B)]
    vT_b = [np.ascontiguousarray(value[b].T) for b in range(B)]

    in_maps = []
    for c in range(NC):
        b, half = c // 2, c % 2
        in_maps.append({
            "qT_in": np.ascontiguousarray(query[b, half * QR:(half + 1) * QR, :].T),
            "kT_in": kT_b[b],
            "vT_in": vT_b[b],
            "WqT": WqT, "WkT": WkT, "WvT": WvT, "WoR": WoR,
            "bq_pf": bq_pf, "bk_pf": bk_pf, "bv_pj": bv_pj, "bo_row": bo_row,
        })

    res = run_bass_kernel_spmd(nc, in_maps, core_ids=list(range(NC)))

    y = np.empty((B, S, D), dtype=np.float32)
    for c in range(NC):
        b, half = c // 2, c % 2
        y[b, half * QR:(half + 1) * QR, :] = res.results[c]["y_out"]
    return y

